# revision 1
# baseline (speedup 1.0000x reference)
"""Multi-head attention (B=2, N=2048, D=1024, H=16) on 8 TRN2 NeuronCores.

Sharding: tensor-parallel over heads. Core c owns heads 2c, 2c+1 (a 128-wide
slice of the concat head dim). Each core:
  - projects Q^T, K^T (transposed layout [dh, rows]) and V (natural [rows, dh])
    for its heads, over all B*N=4096 rows, from host-transposed bf16 x^T inputs
  - attention with transposed scores S^T[k, q] = K Q^T (f32r matmuls), exp on
    ScalarE (scale=1/8 folded in, no max-subtract needed: |scores/8| < ~4),
    softmax denominator via an appended ones-column in V (free on TensorE),
  - partial output projection out^T_c = Wo[:, slice] X_c^T  ->  [1024, 4096]
Host sums the 8 partial outputs and adds bo.

Loop order: batch-0 projections, batch-0 attention, batch-1 projections,
batch-1 attention — so attention starts while the other batch projects.
PV psum accumulators are copied to SBUF immediately (frees the PSUM slot, PE
never idles long enough for the HAM clock-gate to re-throttle); the softmax
normalize chain (reciprocal/broadcast/multiply) runs off the critical path.
"""

import sys

sys.path.insert(0, "/opt/trn_rl_repo")

from contextlib import ExitStack

import ml_dtypes
import numpy as np

import concourse.bass as bass
import concourse.mybir as mybir
import concourse.tile as tile
from concourse import bacc
from concourse.bass_utils import run_bass_kernel_spmd

B, N, D, H, DH = 2, 2048, 1024, 16, 64
R = B * N  # 4096
NC = 8
HPC = H // NC  # 2 heads per core
DHC = HPC * DH  # 128 head dims per core
QT = 512  # query tile (psum bank / fp32 moving max)
KT = 128  # key tile (psum partitions)
NQT = N // QT  # 4
NKT = N // KT  # 16
NBRT = N // QT  # 4 row tiles per batch for projections
KC = D // 128  # 8 contraction chunks

f32 = mybir.dt.float32
f32r = mybir.dt.float32r
bf16 = mybir.dt.bfloat16

_cache = {}


def _fold(ap):
    # [D, X] dram -> [128, KC, X] partition-folded view for one-shot DMA
    return ap.rearrange("(a p) m -> p a m", p=128)


def _foldw(w):
    # [D, DHC] host weight -> [128, KC, DHC] partition-folded, contiguous
    return np.ascontiguousarray(w.reshape(KC, 128, DHC).transpose(1, 0, 2))


def build():
    if "nc" in _cache:
        return _cache["nc"]
    nc = bacc.Bacc("TRN2", target_bir_lowering=False, debug=False, num_devices=NC)
    xq = nc.dram_tensor("xqT", [D, R], bf16, kind="ExternalInput").ap()
    xk = nc.dram_tensor("xkT", [D, R], bf16, kind="ExternalInput").ap()
    xv = nc.dram_tensor("xvT", [D, R], bf16, kind="ExternalInput").ap()
    wq = nc.dram_tensor("wqT", [128, KC, DHC], bf16, kind="ExternalInput").ap()
    wk = nc.dram_tensor("wkT", [128, KC, DHC], bf16, kind="ExternalInput").ap()
    wv = nc.dram_tensor("wvT", [128, KC, DHC], bf16, kind="ExternalInput").ap()
    wo = nc.dram_tensor("woT", [DHC, D], bf16, kind="ExternalInput").ap()
    bq = nc.dram_tensor("bq", [DHC, 1], f32, kind="ExternalInput").ap()
    bk = nc.dram_tensor("bk", [DHC, 1], f32, kind="ExternalInput").ap()
    bv = nc.dram_tensor("bv", [1, DHC], bf16, kind="ExternalInput").ap()
    outT = nc.dram_tensor("outT", [D, R], bf16, kind="ExternalOutput").ap()

    with tile.TileContext(nc) as tc, ExitStack() as ctx:
        const = ctx.enter_context(tc.tile_pool(name="const", bufs=1))
        xpool = ctx.enter_context(tc.tile_pool(name="x", bufs=4))
        big = ctx.enter_context(tc.tile_pool(name="big", bufs=1))
        ppool = ctx.enter_context(tc.tile_pool(name="p", bufs=3))
        opool = ctx.enter_context(tc.tile_pool(name="o", bufs=4))
        npool = ctx.enter_context(tc.tile_pool(name="norm", bufs=3))
        ps_proj = ctx.enter_context(tc.tile_pool(name="psA", bufs=2, space="PSUM"))
        ps_s = ctx.enter_context(tc.tile_pool(name="psS", bufs=2, space="PSUM"))
        ps_pv = ctx.enter_context(tc.tile_pool(name="psPV", bufs=2, space="PSUM"))

        # ---- constants ----
        wq_sb = const.tile([128, KC, DHC], bf16, tag="wq")
        nc.sync.dma_start(wq_sb[:], wq)
        wk_sb = const.tile([128, KC, DHC], bf16, tag="wk")
        nc.sync.dma_start(wk_sb[:], wk)
        wv_sb = const.tile([128, KC, DHC], bf16, tag="wv")
        nc.sync.dma_start(wv_sb[:], wv)
        wo_sb = const.tile([128, D], bf16, tag="wo")
        nc.sync.dma_start(wo_sb[:], wo)
        bq_sb = const.tile([DHC, 1], f32, tag="bq")
        nc.sync.dma_start(bq_sb[:], bq)
        bk_sb = const.tile([DHC, 1], f32, tag="bk")
        nc.sync.dma_start(bk_sb[:], bk)
        bv_sb = const.tile([1, DHC], bf16, tag="bv")
        nc.sync.dma_start(bv_sb[:], bv)
        ones_r = const.tile([1, 128], bf16, tag="onesr")
        nc.vector.memset(ones_r[:], 1.0)

        # ---- per-batch persistent activations ----
        qTs, kTs, vs = [], [], []
        for b in range(B):
            qTs.append(big.tile([128, N], bf16, tag=f"qT{b}", name=f"qT{b}"))
            kTs.append(big.tile([128, N], bf16, tag=f"kT{b}", name=f"kT{b}"))
            v = big.tile([128, HPC * NKT, 128], bf16, tag=f"v{b}", name=f"v{b}")
            nc.vector.memset(v[:, :, 64:128], 1.0)
            vs.append(v)

        XW = 2 * QT  # 1024 rows per x tile (2KB dma descriptors)

        def proj_one(b, dst, xdram, w_sb, b_sb, split=False):
            # psum[dh2, r] = sum_d W^T[d, dh2] x^T[d, r]  (+ bias in the copy)
            for xi in range(N // XW):
                rlo = b * N + xi * XW
                xt = xpool.tile([128, KC, XW], bf16, tag="xqk", name="xqk")
                src_ap = _fold(xdram[:, rlo : rlo + XW])
                if split and xi == 0:
                    for kc in range(KC):
                        nc.sync.dma_start(xt[:, kc, :], src_ap[:, kc, :])
                else:
                    nc.sync.dma_start(xt[:], src_ap)
                for rl in range(XW // QT):
                    ps = ps_proj.tile([128, QT], f32, tag="proj", name="psqk")
                    for kc in range(KC):
                        nc.tensor.matmul(
                            ps[:],
                            w_sb[:, kc, :],
                            xt[:, kc, rl * QT : (rl + 1) * QT],
                            start=(kc == 0), stop=(kc == KC - 1),
                        )
                    rt = xi * (XW // QT) + rl
                    nc.vector.tensor_scalar_add(
                        dst[:, rt * QT : (rt + 1) * QT], ps[:], b_sb[:]
                    )

        def proj_v(b):
            # natural layout: psum[r, dh2] = sum_d x^T[d, r] W^T[d, dh2]
            for xi in range(N // XW):
                rlo = b * N + xi * XW
                xt = xpool.tile([128, KC, XW], bf16, tag="xqk", name="xv")
                nc.sync.dma_start(xt[:], _fold(xv[:, rlo : rlo + XW]))
                for rs in range(XW // 128):
                    ps = ps_proj.tile([128, DHC], f32, tag="proj", name="psv")
                    for kc in range(KC):
                        nc.tensor.matmul(
                            ps[:],
                            xt[:, kc, rs * 128 : (rs + 1) * 128],
                            wv_sb[:, kc, :],
                            start=(kc == 0), stop=False,
                        )
                    nc.tensor.matmul(
                        ps[:], ones_r[:], bv_sb[:], start=False, stop=True
                    )
                    kt = xi * (XW // 128) + rs  # key tile index within batch
                    for h in range(HPC):
                        nc.vector.tensor_copy(
                            vs[b][:, h * NKT + kt, 0:64],
                            ps[:, 64 * h : 64 * h + 64],
                        )

        def proj_k(b):
            proj_one(b, kTs[b], xk, wk_sb, bk_sb, split=(b == 0))

        def proj_q(b):
            proj_one(b, qTs[b], xq, wq_sb, bq_sb)

        def attention(b, xT, qts):
            for qt in qts:
                qs = slice(qt * QT, (qt + 1) * QT)
                pvs = [
                    ps_pv.tile([128, QT], f32, tag="pv", name=f"pv{h}")
                    for h in range(HPC)
                ]
                for kt in range(NKT):
                    ks = slice(kt * KT, (kt + 1) * KT)
                    sg = ps_s.tile([128, 2 * QT], f32, tag="sg", name="sg")
                    for h in range(HPC):
                        hp = slice(64 * h, 64 * h + 64)
                        nc.tensor.matmul(
                            sg[:, h * QT : (h + 1) * QT],
                            kTs[b][hp, ks],
                            qTs[b][hp, qs],
                            start=True, stop=True,
                        )
                    pt = ppool.tile([128, 2 * QT], bf16, tag="p", name="pt")
                    nc.scalar.activation(
                        pt[:], sg[:], mybir.ActivationFunctionType.Exp, scale=0.125
                    )
                    for h in range(HPC):
                        nc.tensor.matmul(
                            pvs[h][:],
                            vs[b][:, h * NKT + kt, :],
                            pt[:, h * QT : (h + 1) * QT],
                            start=(kt == 0), stop=(kt == NKT - 1),
                        )
                for h in range(HPC):
                    # copy to SBUF immediately -> frees the psum slot so the
                    # next q-tile's PV starts without a long PE stall
                    pvsb = npool.tile([65, QT], f32, tag="pvsb", name=f"pvsb{h}")
                    nc.vector.tensor_copy(pvsb[:], pvs[h][0:65, :])
                    # sumexp row sits at partition 64; shift to 0 via sbuf DMA
                    rc = npool.tile([1, QT], f32, tag="rc", name=f"rc{h}")
                    nc.sync.dma_start(rc[:], pvsb[64:65, :])
                    nc.vector.reciprocal_approx_fast(rc[:], rc[:])
                    rb = npool.tile([64, QT], f32, tag="rb", name=f"rb{h}")
                    nc.gpsimd.partition_broadcast(rb[:], rc[:])
                    if h == 0:
                        nc.vector.tensor_mul(xT[0:64, qs], pvsb[0:64, :], rb[:])
                    else:
                        tmp = npool.tile([64, QT], bf16, tag="tmp", name="tmp")
                        nc.vector.tensor_mul(tmp[:], pvsb[0:64, :], rb[:])
                        nc.sync.dma_start(xT[64:128, qs], tmp[:])
                last = b == B - 1 and qt == NQT - 1
                for ot in range(KC):
                    ps = ps_proj.tile([128, QT], f32, tag="proj", name="pso")
                    nc.tensor.matmul(
                        ps[:],
                        wo_sb[:, ot * 128 : (ot + 1) * 128],
                        xT[:, qs],
                        start=True, stop=True,
                    )
                    ob = opool.tile([128, QT], bf16, tag="o", name="ob")
                    if last and ot % 2 == 0:
                        nc.scalar.copy(ob[:], ps[:])
                    else:
                        nc.vector.tensor_copy(ob[:], ps[:])
                    nc.gpsimd.dma_start(
                        outT[
                            ot * 128 : (ot + 1) * 128,
                            b * N + qt * QT : b * N + (qt + 1) * QT,
                        ],
                        ob[:],
                    )

        xTs = [
            opool.tile([128, N], bf16, tag="xT", name=f"xT{b}", bufs=2)
            for b in range(B)
        ]
        proj_k(0)
        proj_v(0)
        proj_q(0)
        attention(0, xTs[0], range(0, 1))
        proj_k(1)
        attention(0, xTs[0], range(1, 2))
        proj_v(1)
        attention(0, xTs[0], range(2, 3))
        proj_q(1)
        attention(0, xTs[0], range(3, NQT))
        attention(1, xTs[1], range(0, NQT))

    nc.compile()
    _cache["nc"] = nc
    return nc


def kernel(x_q, x_k, x_v, Wq, bq, Wk, bk, Wv, bv, Wo, bo, _trace=False):
    x_q = np.asarray(x_q, dtype=np.float32)
    x_k = np.asarray(x_k, dtype=np.float32)
    x_v = np.asarray(x_v, dtype=np.float32)
    Wq, Wk, Wv, Wo = (np.asarray(w, dtype=np.float32) for w in (Wq, Wk, Wv, Wo))
    bq, bk, bv, bo = (np.asarray(v, dtype=np.float32) for v in (bq, bk, bv, bo))

    bf = ml_dtypes.bfloat16
    xqT = np.ascontiguousarray(x_q.reshape(R, D).T).astype(bf)
    xkT = np.ascontiguousarray(x_k.reshape(R, D).T).astype(bf)
    xvT = np.ascontiguousarray(x_v.reshape(R, D).T).astype(bf)

    in_maps = []
    for c in range(NC):
        s = slice(DHC * c, DHC * (c + 1))
        in_maps.append(
            {
                "xqT": xqT,
                "xkT": xkT,
                "xvT": xvT,
                "wqT": _foldw(Wq[s, :].T).astype(bf),
                "wkT": _foldw(Wk[s, :].T).astype(bf),
                "wvT": _foldw(Wv[s, :].T).astype(bf),
                "woT": np.ascontiguousarray(Wo[:, s].T).astype(bf),
                "bq": bq[s][:, None].copy(),
                "bk": bk[s][:, None].copy(),
                "bv": bv[s][None, :].astype(bf),
            }
        )

    nc = build()
    res = run_bass_kernel_spmd(nc, in_maps, core_ids=list(range(NC)), trace=_trace)
    total = np.zeros((D, R), dtype=np.float32)
    for c in range(NC):
        total += res.results[c]["outT"].astype(np.float32)
    out = total.T + bo[None, :]
    if _trace:
        kernel.last_exec_time_ns = res.exec_time_ns
    return out.reshape(B, N, D).astype(np.float32)



# revision 9
# speedup vs baseline: 1.0853x; 1.0853x over previous
"""Multi-head attention (B=2, N=2048, D=1024, H=16) on 8 TRN2 NeuronCores.

Sharding: tensor-parallel over heads. Core c owns heads 2c, 2c+1 (a 128-wide
slice of the concat head dim). Each core:
  - projects Q^T, K^T, V^T (transposed layout [dh, rows]) for its heads over
    all B*N=4096 rows from host-transposed bf16 x^T inputs (W stationary,
    x moving -> all matmuls stream 512 moving cols)
  - V^T is DMA-transposed (XBAR) into natural [keys, dh] tiles with an
    appended ones-column per head (sumexp rides the PV matmul for free)
  - attention with transposed scores S^T[k, q] = K Q^T, exp on ScalarE
    (scale=1/8 folded in; no max-subtract needed: |scores/8| < ~4)
  - partial output projection out^T_c = Wo[:, slice] X_c^T -> [1024, 4096]
Host sums the 8 partial outputs and adds bo.

Schedule: a single software-pipelined instruction stream. The attention kt
loop is scalar-bound (exp ~1.1us per kt vs ~0.86us of PE work), so all other
PE work (projections of the other batch, output projections, PE warm-up) is
queued as "filler" matmuls and pumped between the scores and PV matmuls of
each kt iteration, keeping the PE continuously busy (which also keeps the
HAM clock-gate at full rate). PSUM: sg ring 2x2 banks + pv ring 2 banks +
proj/out ring 2 banks = 8 banks exactly.
"""

import sys

sys.path.insert(0, "/opt/trn_rl_repo")

from contextlib import ExitStack

import ml_dtypes
import numpy as np

import concourse.bass as bass
import concourse.mybir as mybir
import concourse.tile as tile
from concourse import bacc, masks
from concourse.bass_utils import run_bass_kernel_spmd

B, N, D, H, DH = 2, 2048, 1024, 16, 64
R = B * N  # 4096
NC = 8
HPC = H // NC  # 2 heads per core
DHC = HPC * DH  # 128 head dims per core
QT = 512  # query tile (psum bank of fp32)
KT = 128  # key tile (psum partitions)
NQT = N // QT  # 4
NKT = N // KT  # 16
KC = D // 128  # 8 contraction chunks
XW = 1024  # rows per x tile

f32 = mybir.dt.float32
bf16 = mybir.dt.bfloat16

_cache = {}


def _fold(ap):
    # [D, X] dram -> [128, KC, X] partition-folded view for one-shot DMA
    return ap.rearrange("(a p) m -> p a m", p=128)


def _foldw(w):
    # [D, DHC] host weight -> [128, KC, DHC] partition-folded, contiguous
    return np.ascontiguousarray(w.reshape(KC, 128, DHC).transpose(1, 0, 2))


def build():
    if "nc" in _cache:
        return _cache["nc"]
    nc = bacc.Bacc("TRN2", target_bir_lowering=False, debug=False, num_devices=NC)
    xq = nc.dram_tensor("xqT", [D, R], bf16, kind="ExternalInput").ap()
    xk = nc.dram_tensor("xkT", [D, R], bf16, kind="ExternalInput").ap()
    xv = nc.dram_tensor("xvT", [D, R], bf16, kind="ExternalInput").ap()
    wq = nc.dram_tensor("wqT", [128, KC, DHC], bf16, kind="ExternalInput").ap()
    wk = nc.dram_tensor("wkT", [128, KC, DHC], bf16, kind="ExternalInput").ap()
    wv = nc.dram_tensor("wvT", [128, KC, DHC], bf16, kind="ExternalInput").ap()
    wo = nc.dram_tensor("woT", [DHC, D], bf16, kind="ExternalInput").ap()
    bq = nc.dram_tensor("bq", [DHC, 1], f32, kind="ExternalInput").ap()
    bk = nc.dram_tensor("bk", [DHC, 1], f32, kind="ExternalInput").ap()
    bv = nc.dram_tensor("bv", [DHC, 1], f32, kind="ExternalInput").ap()
    outT = nc.dram_tensor("outT", [D, R], bf16, kind="ExternalOutput").ap()

    with tile.TileContext(nc) as tc, ExitStack() as ctx:
        const = ctx.enter_context(tc.tile_pool(name="const", bufs=1))
        xpool = ctx.enter_context(tc.tile_pool(name="x", bufs=3))
        big = ctx.enter_context(tc.tile_pool(name="big", bufs=1))
        ppool = ctx.enter_context(tc.tile_pool(name="p", bufs=3))
        opool = ctx.enter_context(tc.tile_pool(name="o", bufs=4))
        npool = ctx.enter_context(tc.tile_pool(name="norm", bufs=2))
        ps_proj = ctx.enter_context(tc.tile_pool(name="psA", bufs=2, space="PSUM"))
        ps_s = ctx.enter_context(tc.tile_pool(name="psS", bufs=2, space="PSUM"))
        ps_pv = ctx.enter_context(tc.tile_pool(name="psPV", bufs=2, space="PSUM"))

        # ---- local SBUF constants / persistent tiles ----
        warm = const.tile([128, QT], bf16, tag="warm")
        nc.gpsimd.memset(warm[:], 0.0)
        ident = const.tile([128, 128], bf16, tag="ident")
        masks.make_identity(nc, ident[:])

        wk_sb = const.tile([128, KC, DHC], bf16, tag="wk")
        bk_sb = const.tile([DHC, 1], f32, tag="bk")
        wv_sb = const.tile([128, KC, DHC], bf16, tag="wv")
        bv_sb = const.tile([DHC, 1], f32, tag="bv")
        wq_sb = const.tile([128, KC, DHC], bf16, tag="wq")
        bq_sb = const.tile([DHC, 1], f32, tag="bq")
        wo_sb = const.tile([128, D], bf16, tag="wo")

        qTs, kTs, vTs, vs, xTs = [], [], [], [], []
        for b in range(B):
            qTs.append(big.tile([128, N], bf16, tag=f"qT{b}", name=f"qT{b}"))
            kTs.append(big.tile([128, N], bf16, tag=f"kT{b}", name=f"kT{b}"))
            vTs.append(big.tile([128, N], bf16, tag=f"vT{b}", name=f"vT{b}"))
            v = big.tile([128, NKT, HPC, 65], bf16, tag=f"v{b}", name=f"v{b}")
            nc.gpsimd.memset(v[:, :, :, 64:65], 1.0)
            vs.append(v)
            xTs.append(opool.tile([128, N], bf16, tag=f"xT{b}", name=f"xT{b}"))

        # ---- weight / first-x DMA issue order (sync HWDGE queue) ----
        nc.sync.dma_start(wk_sb[:], wk)
        nc.sync.dma_start(bk_sb[:], bk)
        nc.sync.dma_start(wv_sb[:], wv)
        nc.sync.dma_start(bv_sb[:], bv)
        nc.sync.dma_start(wq_sb[:], wq)
        nc.sync.dma_start(bq_sb[:], bq)
        nc.sync.dma_start(wo_sb[:], wo)

        # ---- PE warm-up: garbage matmuls to lift the HAM clock-gate while
        # the first DMAs land (outputs never read) ----
        for i in range(8):
            psw = ps_proj.tile([128, QT], f32, tag="proj", name="psw")
            nc.tensor.matmul(psw[:], warm[:, 0:128], warm[:], start=True, stop=True)

        # ---- preload the exp activation table while ScalarE is idle ----
        junk = const.tile([128, 1], bf16, tag="junk")
        nc.scalar.activation(
            junk[:], warm[:, 0:1], mybir.ActivationFunctionType.Exp, scale=0.125
        )

        # ================= filler machinery =================
        fill_q = []

        def pump(ns):
            while fill_q and ns > 0:
                ns -= fill_q.pop(0)()
            # flush any zero-cost ops at the head
            while fill_q and fill_q[0].__dict__.get("free", False):
                fill_q.pop(0)()

        def flush():
            while fill_q:
                fill_q.pop(0)()

        def run_gen(g):
            # immediate (non-filler) execution of a micro-op generator
            for f in g:
                f()

        def free_op(f):
            def g():
                f()
                return 0
            g.free = True
            return g

        # ---- projection generator: dst[dh, rows] = W^T x^T (+ bias) ----
        def gen_proj(b, dst, xdram, w_sb, b_sb, split, transpose_v=None):
            for xi in range(N // XW):
                rlo = b * N + xi * XW
                xt = xpool.tile([128, KC, XW], bf16, tag="x", name="xt")
                src = _fold(xdram[:, rlo : rlo + XW])
                if split:
                    yield free_op(lambda xt=xt, src=src: nc.sync.dma_start(
                        xt[:, 0:1, :], src[:, 0:1, :]
                    ))
                    yield free_op(lambda xt=xt, src=src: nc.sync.dma_start(
                        xt[:, 1:KC, :], src[:, 1:KC, :]
                    ))
                else:
                    yield free_op(lambda xt=xt, src=src: nc.sync.dma_start(xt[:], src))
                for rl in range(XW // QT):
                    ps = ps_proj.tile([128, QT], f32, tag="proj", name="psp")
                    for kc in range(KC):
                        def mm(ps=ps, xt=xt, kc=kc, rl=rl):
                            nc.tensor.matmul(
                                ps[:],
                                w_sb[:, kc, :],
                                xt[:, kc, rl * QT : (rl + 1) * QT],
                                start=(kc == 0), stop=(kc == KC - 1),
                            )
                            return 215
                        yield mm
                    rt = xi * (XW // QT) + rl
                    def badd(ps=ps, rt=rt):
                        nc.vector.tensor_scalar_add(
                            dst[:, rt * QT : (rt + 1) * QT], ps[:], b_sb[:]
                        )
                        return 0
                    yield badd
                    if transpose_v is not None:
                        vtile = transpose_v
                        # the 512-row chunk covers 4 key tiles; PE-transpose
                        # each [128 dh, 128 keys] block to natural V layout
                        for kt in range(rt * 4, rt * 4 + 4):
                            pst = ps_proj.tile(
                                [128, 128], bf16, tag="proj", name="tp"
                            )
                            def tr(kt=kt, pst=pst):
                                nc.tensor.transpose(
                                    pst[:],
                                    dst[:, kt * 128 : (kt + 1) * 128],
                                    ident[:],
                                )
                                return 215
                            yield tr
                            def trc(kt=kt, pst=pst, vtile=vtile):
                                nc.vector.tensor_copy(
                                    vtile[:, kt, :, 0:64],
                                    pst[:].rearrange("p (h c) -> p h c", h=HPC),
                                )
                                return 0
                            yield trc

        # ---- output projection generator for one (b, qt) ----
        def gen_outproj(b, qt, tail=False):
            qs = slice(qt * QT, (qt + 1) * QT)
            for ot in range(KC):
                ps = ps_proj.tile([128, QT], f32, tag="proj", name="pso")
                def mm(ps=ps, ot=ot, b=b):
                    nc.tensor.matmul(
                        ps[:],
                        wo_sb[:, ot * 128 : (ot + 1) * 128],
                        xTs[b][:, qs],
                        start=True, stop=True,
                    )
                    return 215
                yield mm
                ob = opool.tile([128, QT], bf16, tag="o", name="ob")
                def cst(ps=ps, ob=ob, ot=ot):
                    if tail and ot % 2 == 0:
                        nc.scalar.copy(ob[:], ps[:])
                    else:
                        nc.vector.tensor_copy(ob[:], ps[:])
                    return 0
                yield cst
                def st(ob=ob, ot=ot, b=b, qt=qt):
                    dst = outT[
                        ot * 128 : (ot + 1) * 128,
                        b * N + qt * QT : b * N + (qt + 1) * QT,
                    ]
                    if tail:
                        nc.sync.dma_start(dst, ob[:])
                    else:
                        nc.gpsimd.dma_start(dst, ob[:])
                    return 0
                yield st

        # ---- attention for one (b, qt): software-pipelined kt loop ----
        def attention_qt(b, qt, budget):
            qs = slice(qt * QT, (qt + 1) * QT)
            pvs = [
                ps_pv.tile([65, QT], f32, tag="pv", name=f"pv{h}")
                for h in range(HPC)
            ]
            sgs, pts = {}, {}

            def scores(kt):
                sg = ps_s.tile([128, 2 * QT], f32, tag="sg", name="sg")
                sgs[kt] = sg
                ks = slice(kt * KT, (kt + 1) * KT)
                for h in range(HPC):
                    hp = slice(64 * h, 64 * h + 64)
                    nc.tensor.matmul(
                        sg[:, h * QT : (h + 1) * QT],
                        kTs[b][hp, ks],
                        qTs[b][hp, qs],
                        start=True, stop=True,
                    )

            def expx(kt):
                pt = ppool.tile([128, 2 * QT], bf16, tag="pt", name="pt")
                pts[kt] = pt
                nc.scalar.activation(
                    pt[:], sgs.pop(kt)[:],
                    mybir.ActivationFunctionType.Exp, scale=0.125,
                )

            def pv(kt):
                pt = pts.pop(kt)
                for h in range(HPC):
                    nc.tensor.matmul(
                        pvs[h][:],
                        vs[b][:, kt, h, :],
                        pt[:, h * QT : (h + 1) * QT],
                        start=(kt == 0), stop=(kt == NKT - 1),
                    )

            scores(0)
            expx(0)
            for kt in range(NKT):
                if kt + 1 < NKT:
                    scores(kt + 1)
                    expx(kt + 1)
                pump(budget)
                pv(kt)
            return pvs

        # ---- normalize X = PV / sumexp into xTs[b][:, qs] ----
        def normalize(b, qt, pvs):
            qs = slice(qt * QT, (qt + 1) * QT)
            for h in range(HPC):
                pvsb = npool.tile([65, QT], f32, tag="pvsb", name=f"pvsb{h}")
                nc.vector.tensor_copy(pvsb[:], pvs[h][:])
                rc = npool.tile([1, QT], f32, tag="rc", name=f"rc{h}")
                nc.sync.dma_start(rc[:], pvsb[64:65, :])
                nc.vector.reciprocal_approx_fast(rc[:], rc[:])
                rb = npool.tile([64, QT], f32, tag="rb", name=f"rb{h}")
                nc.gpsimd.partition_broadcast(rb[:], rc[:])
                if h == 0:
                    nc.vector.tensor_mul(xTs[b][0:64, qs], pvsb[0:64, :], rb[:])
                else:
                    tmp = npool.tile([64, QT], bf16, tag="tmp", name="tmp")
                    nc.vector.tensor_mul(tmp[:], pvsb[0:64, :], rb[:])
                    nc.sync.dma_start(xTs[b][64:128, qs], tmp[:])

        # ================= the schedule =================
        # batch-0 projections: K, V (+transposes on the idle scalar HWDGE), Q
        run_gen(gen_proj(0, kTs[0], xk, wk_sb, bk_sb, split=True))
        run_gen(gen_proj(0, vTs[0], xv, wv_sb, bv_sb, split=True,
                         transpose_v=vs[0]))
        qgen = gen_proj(0, qTs[0], xq, wq_sb, bq_sb, split=True)
        # emit only the first 512-row chunk of Q (first 11 micro-ops:
        # dma, dma, 8 matmuls, bias-add), then queue the rest as fillers
        for _ in range(11):
            next(qgen)()
        fill_q.extend(qgen)

        # batch-1 projections become fillers consumed during attention(0)
        fill_q.extend(gen_proj(1, kTs[1], xk, wk_sb, bk_sb, split=False))
        fill_q.extend(gen_proj(1, vTs[1], xv, wv_sb, bv_sb, split=False,
                               transpose_v=vs[1]))
        fill_q.extend(gen_proj(1, qTs[1], xq, wq_sb, bq_sb, split=False))

        for qt in range(NQT):
            pvs = attention_qt(0, qt, budget=600)
            normalize(0, qt, pvs)
            if qt > 0:
                fill_q.extend(gen_outproj(0, qt - 1))
        flush()
        fill_q.extend(gen_outproj(0, NQT - 1))

        for qt in range(NQT):
            pvs = attention_qt(1, qt, budget=450)
            normalize(1, qt, pvs)
            if qt > 0:
                fill_q.extend(gen_outproj(1, qt - 1))
        flush()
        run_gen(gen_outproj(1, NQT - 1, tail=True))

    nc.compile()
    _cache["nc"] = nc
    return nc


def kernel(x_q, x_k, x_v, Wq, bq, Wk, bk, Wv, bv, Wo, bo, _trace=False):
    x_q = np.asarray(x_q, dtype=np.float32)
    x_k = np.asarray(x_k, dtype=np.float32)
    x_v = np.asarray(x_v, dtype=np.float32)
    Wq, Wk, Wv, Wo = (np.asarray(w, dtype=np.float32) for w in (Wq, Wk, Wv, Wo))
    bq, bk, bv, bo = (np.asarray(v, dtype=np.float32) for v in (bq, bk, bv, bo))

    bf = ml_dtypes.bfloat16
    xqT = np.ascontiguousarray(x_q.reshape(R, D).T).astype(bf)
    xkT = np.ascontiguousarray(x_k.reshape(R, D).T).astype(bf)
    xvT = np.ascontiguousarray(x_v.reshape(R, D).T).astype(bf)

    in_maps = []
    for c in range(NC):
        s = slice(DHC * c, DHC * (c + 1))
        in_maps.append(
            {
                "xqT": xqT,
                "xkT": xkT,
                "xvT": xvT,
                "wqT": _foldw(Wq[s, :].T).astype(bf),
                "wkT": _foldw(Wk[s, :].T).astype(bf),
                "wvT": _foldw(Wv[s, :].T).astype(bf),
                "woT": np.ascontiguousarray(Wo[:, s].T).astype(bf),
                "bq": bq[s][:, None].copy(),
                "bk": bk[s][:, None].copy(),
                "bv": bv[s][:, None].copy(),
            }
        )

    nc = build()
    res = run_bass_kernel_spmd(nc, in_maps, core_ids=list(range(NC)), trace=_trace)
    total = np.zeros((D, R), dtype=np.float32)
    for c in range(NC):
        total += res.results[c]["outT"].astype(np.float32)
    out = total.T + bo[None, :]
    if _trace:
        kernel.last_exec_time_ns = res.exec_time_ns
    return out.reshape(B, N, D).astype(np.float32)
